# revision 1
# baseline (speedup 1.0000x reference)
import numpy as np
import jax
import jax.numpy as jnp
from functools import partial

# nn_Block_81088982549089: dense transformer block
# x: (16384, 64, 16) fp32. Pure data parallel over 8 cores (batch dim).
D_MODEL = 16
D_FF = 32
BLOCK = 64
B = 16384
EPS = 1e-5
N_CORES = 8


def _layernorm(x, g, b):
    mu = jnp.mean(x, axis=-1, keepdims=True)
    var = jnp.mean(jnp.square(x - mu), axis=-1, keepdims=True)
    return (x - mu) * jax.lax.rsqrt(var + EPS) * g + b


def _block_fwd(x, Wk, Wq, Wv, ln1_g, ln1_b, ln2_g, ln2_b, W1, b1, W2, b2):
    B_, T, C = x.shape
    h = _layernorm(x, ln1_g, ln1_b)
    k = h @ Wk
    q = h @ Wq
    v = h @ Wv
    att = jnp.einsum("btc,bsc->bts", q, k) / jnp.sqrt(jnp.float32(C))
    causal = jnp.tril(jnp.ones((T, T), dtype=bool))
    att = jnp.where(causal, att, -jnp.inf)
    att = jax.nn.softmax(att, axis=-1)
    y = jnp.einsum("bts,bsc->btc", att, v)
    x = x + y
    h2 = _layernorm(x, ln2_g, ln2_b)
    ff = jax.nn.gelu(h2 @ W1 + b1, approximate=False) @ W2 + b2
    return x + ff


_PMAPPED = None


def _get_pmapped():
    global _PMAPPED
    if _PMAPPED is None:
        _PMAPPED = jax.pmap(_block_fwd, axis_name="i",
                            in_axes=(0,) + (None,) * 11)
    return _PMAPPED


def kernel(x, Wk, Wq, Wv, ln1_g, ln1_b, ln2_g, ln2_b, W1, b1, W2, b2):
    x = np.asarray(x, dtype=np.float32)
    xs = x.reshape(N_CORES, B // N_CORES, BLOCK, D_MODEL)
    f = _get_pmapped()
    out = f(xs, np.asarray(Wk, np.float32), np.asarray(Wq, np.float32),
            np.asarray(Wv, np.float32), np.asarray(ln1_g, np.float32),
            np.asarray(ln1_b, np.float32), np.asarray(ln2_g, np.float32),
            np.asarray(ln2_b, np.float32), np.asarray(W1, np.float32),
            np.asarray(b1, np.float32), np.asarray(W2, np.float32),
            np.asarray(b2, np.float32))
    out = np.asarray(out)
    return out.reshape(B, BLOCK, D_MODEL)
